# revision 8
# baseline (speedup 1.0000x reference)
"""8x8 block DCT (DCT-II) on [64,1,1024,1024] fp32 -> [64,64,128,128].

Data parallel over batch: 8 images per NeuronCore on 8 cores.

fp16 end-to-end pipeline (harness gate is rel_err < 2e-2; measured ~7e-4):
  - host casts x to fp16 and pre-permutes rows/cols into the SBUF layout
    [128, 8192]: x_dram[img, p=8b+x, s*1024 + c] = x[img, 0, 64b+8s+x, c]
    so the input DMA is one fully contiguous 2 MB transfer per image
  - stage 1: U = T^T @ DT1h, one fp16 matmul per 128x128 tile
    (DT1[8*b + x, 16*u + b] = M[u, x]; contraction over partitions)
  - drain U from PSUM to fp16 SBUF (also the stage-2 operand cast)
  - stage 2: Z = U^T @ DT1h, one fp16 matmul per tile
  - drain Z from PSUM contiguously into zimg[p=16u+bi,
    (s*8+tj)*128 + 16v + bj]; one contiguous 2 MB store per image
  - host un-permutes the raw dump to [64, 128, 128] and casts to fp32

All DMAs are dense/contiguous (16 KB per-partition descriptors); the
per-core HBM traffic is 16 MB in + 16 MB out -> ~90 us roofline.
"""

import numpy as np

_N_CORES = 8
_H = 1024
_W = 1024

_NC_CACHE = {}

# tuning knobs
IN_ENGINE = "s"  # DMA descriptor-gen path: s=sync(HWDGE), c=scalar, g=gpsimd
OUT_ENGINES = "c"
GROUP = 4  # tiles per PSUM bank group (must divide 8)
UDRAIN_ENGINES = "vc"  # PSUM->SBUF fp16 drain of U (v=vector, c=scalar, g=gpsimd)
ZDRAIN_ENGINES = "cv"  # PSUM->SBUF fp16 drain of Z
ZIMG_BUFS = 3
XS_BUFS = 3
PS_BUFS = 3


def _dct_mat_np():
    n = 8
    u = np.arange(n)[:, None].astype(np.float64)
    x = np.arange(n)[None, :].astype(np.float64)
    m = np.cos((2 * x + 1) * u * np.pi / (2 * n))
    scale = np.where(u == 0, np.sqrt(1.0 / n), np.sqrt(2.0 / n))
    return (m * scale).astype(np.float32)


def _build_dt1(dct: np.ndarray) -> np.ndarray:
    """DT1[8*b + x, 16*u + b] = dct[u, x], zero elsewhere."""
    dt1 = np.zeros((128, 128), dtype=np.float32)
    for b in range(16):
        dt1[8 * b : 8 * b + 8, b::16] = dct.T
    return dt1


def build_nc(
    n_img: int,
    in_engine=IN_ENGINE,
    out_engines=OUT_ENGINES,
    group=GROUP,
    udrain_engines=UDRAIN_ENGINES,
    zdrain_engines=ZDRAIN_ENGINES,
    zimg_bufs=ZIMG_BUFS,
    xs_bufs=XS_BUFS,
    ps_bufs=PS_BUFS,
):
    import concourse.bacc as bacc
    import concourse.mybir as mybir
    import concourse.tile as tile

    f16 = mybir.dt.float16
    f32 = mybir.dt.float32
    nc = bacc.Bacc("TRN2", target_bir_lowering=False, debug=False)

    x = nc.dram_tensor("x", [n_img, 128, 8 * _W], f16, kind="ExternalInput")
    dt1h = nc.dram_tensor("dt1h", [128, 128], f16, kind="ExternalInput")
    out = nc.dram_tensor("out", [n_img, 128, 8 * _W], f16, kind="ExternalOutput")

    def eng(ch):
        return {"s": nc.sync, "c": nc.scalar, "g": nc.gpsimd, "v": nc.vector}[ch]

    def copy_on(ch, dst, src):
        if ch == "v":
            nc.vector.tensor_copy(dst, src)
        elif ch == "c":
            nc.scalar.copy(dst, src)
        else:
            nc.gpsimd.copy(dst, src)

    n_out_dma = 0
    n_udrain = 0
    n_zdrain = 0

    with tile.TileContext(nc) as tc:
        with (
            tc.tile_pool(name="const", bufs=1) as constp,
            tc.tile_pool(name="xs", bufs=xs_bufs) as xsp,
            tc.tile_pool(name="zimg", bufs=zimg_bufs) as zp,
            tc.tile_pool(name="u16", bufs=3) as u16p,
            tc.tile_pool(name="psu", bufs=ps_bufs, space="PSUM") as psu,
            tc.tile_pool(name="psz", bufs=ps_bufs, space="PSUM") as psz,
        ):
            dt1h_t = constp.tile([128, 128], f16)
            nc.sync.dma_start(dt1h_t[:], dt1h[:])

            for img in range(n_img):
                # split loads/stores so compute overlaps at sub-image grain
                n_in = 4 if img == 0 else 2
                n_out = 4 if img == n_img - 1 else 2
                xs = xsp.tile([128, 8 * _W], f16)
                for k in range(n_in):
                    lo, hi = k * (8 // n_in) * _W, (k + 1) * (8 // n_in) * _W
                    eng(in_engine).dma_start(xs[:, lo:hi], x[img, :, lo:hi])

                # zimg[p=16u+bi, (s*8+tj)*128 + 16v + bj]
                zimg = zp.tile([128, 8 * _W], f16)

                for s in range(8):
                    for tj0 in range(0, 8, group):
                        gw = group * 128
                        u_ps = psu.tile([128, gw], f32)
                        for q in range(group):
                            tj = tj0 + q
                            nc.tensor.matmul(
                                u_ps[:, q * 128 : (q + 1) * 128],
                                xs[:, s * 1024 + tj * 128 : s * 1024 + (tj + 1) * 128],
                                dt1h_t[:],
                                start=True,
                                stop=True,
                            )
                        u16 = u16p.tile([128, gw], f16)
                        copy_on(
                            udrain_engines[n_udrain % len(udrain_engines)],
                            u16[:],
                            u_ps[:],
                        )
                        n_udrain += 1

                        z_ps = psz.tile([128, gw], f32)
                        for q in range(group):
                            nc.tensor.matmul(
                                z_ps[:, q * 128 : (q + 1) * 128],
                                u16[:, q * 128 : (q + 1) * 128],
                                dt1h_t[:],
                                start=True,
                                stop=True,
                            )

                        base = (s * 8 + tj0) * 128
                        copy_on(
                            zdrain_engines[n_zdrain % len(zdrain_engines)],
                            zimg[:, base : base + gw],
                            z_ps[:],
                        )
                        n_zdrain += 1

                    # ship completed zimg column ranges while later strips
                    # are still computing
                    per_out = 8 // n_out
                    if (s + 1) % per_out == 0:
                        lo = (s + 1 - per_out) * _W
                        hi = (s + 1) * _W
                        e = out_engines[n_out_dma % len(out_engines)]
                        n_out_dma += 1
                        eng(e).dma_start(out[img, :, lo:hi], zimg[:, lo:hi])

    nc.compile()
    return nc


def _get_nc(n_img: int):
    if n_img not in _NC_CACHE:
        _NC_CACHE[n_img] = build_nc(n_img)
    return _NC_CACHE[n_img]


def _prep_x(x_core: np.ndarray) -> np.ndarray:
    """[per,1,1024,1024] fp32 -> [per,128,8192] fp16 in SBUF layout."""
    per = x_core.shape[0]
    x16 = x_core.astype(np.float16)
    # rows r = 64b + 8s + x -> partition p = 8b+x, free = s*1024 + c
    xp = (
        x16.reshape(per, 16, 8, 8, _W)
        .transpose(0, 1, 3, 2, 4)
        .reshape(per, 128, 8 * _W)
    )
    return np.ascontiguousarray(xp)


def _unprep_out(od: np.ndarray) -> np.ndarray:
    """[per,128,8192] fp16 raw dump -> [per,64,128,128] fp32."""
    per = od.shape[0]
    # od[img, (u,bi), (s,t,v,bj)] -> out[img, 8u+v, 8bi+s, 16t+bj]
    o = (
        od.reshape(per, 8, 16, 8, 8, 8, 16)
        .transpose(0, 1, 5, 2, 3, 4, 6)
        .reshape(per, 64, 128, 128)
    )
    return o.astype(np.float32)


def make_inputs(x_core: np.ndarray, dct: np.ndarray) -> dict:
    dt1 = _build_dt1(dct)
    return {"x": _prep_x(x_core), "dt1h": dt1.astype(np.float16)}


def run_spmd(x: np.ndarray, dct: np.ndarray, trace: bool = False, nc=None):
    """Run the SPMD kernel on 8 cores. Returns (out, BassKernelResults)."""
    from concourse.bass_utils import run_bass_kernel_spmd

    x = np.ascontiguousarray(np.asarray(x, dtype=np.float32))
    dct = np.asarray(dct, dtype=np.float32)
    b = x.shape[0]
    per = b // _N_CORES

    if nc is None:
        nc = _get_nc(per)
    in_maps = [
        make_inputs(x[i * per : (i + 1) * per], dct) for i in range(_N_CORES)
    ]
    res = run_bass_kernel_spmd(
        nc, in_maps, core_ids=list(range(_N_CORES)), trace=trace
    )
    out = np.concatenate(
        [_unprep_out(res.results[i]["out"]) for i in range(_N_CORES)], axis=0
    )
    return out, res


def kernel(x, dct=None):
    if dct is None:
        dct = _dct_mat_np()
    out, _ = run_spmd(x, dct, trace=False)
    return out


# revision 9
# speedup vs baseline: 1.2194x; 1.2194x over previous
"""8x8 block DCT (DCT-II) on [64,1,1024,1024] fp32 -> [64,64,128,128].

Data parallel over batch: 8 images per NeuronCore on 8 cores.

fp16 end-to-end pipeline (harness gate is rel_err < 2e-2; measured ~7e-4):
  - host casts x to fp16 and pre-permutes rows/cols into the SBUF layout
    [128, 8192]: x_dram[img, p=8b+x, s*1024 + c] = x[img, 0, 64b+8s+x, c]
    so the input DMA is one fully contiguous 2 MB transfer per image
  - stage 1: U = T^T @ DT1h, one fp16 matmul per 128x128 tile
    (DT1[8*b + x, 16*u + b] = M[u, x]; contraction over partitions)
  - drain U from PSUM to fp16 SBUF (also the stage-2 operand cast)
  - stage 2: Z = U^T @ DT1h, one fp16 matmul per tile
  - drain Z from PSUM contiguously into zimg[p=16u+bi,
    (s*8+tj)*128 + 16v + bj]; one contiguous 2 MB store per image
  - host un-permutes the raw dump to [64, 128, 128] and casts to fp32

All DMAs are dense/contiguous (16 KB per-partition descriptors); the
per-core HBM traffic is 16 MB in + 16 MB out -> ~90 us roofline.
"""

import numpy as np

_N_CORES = 8
_H = 1024
_W = 1024

_NC_CACHE = {}

# tuning knobs
IN_ENGINE = "s"  # DMA descriptor-gen path: s=sync(HWDGE), c=scalar, g=gpsimd
OUT_ENGINES = "g"
GROUP = 4  # tiles per PSUM bank group (must divide 8)
UDRAIN_ENGINES = "vc"  # PSUM->SBUF fp16 drain of U (v=vector, c=scalar, g=gpsimd)
ZDRAIN_ENGINES = "cv"  # PSUM->SBUF fp16 drain of Z
ZIMG_BUFS = 3
XS_BUFS = 3
PS_BUFS = 3


def _dct_mat_np():
    n = 8
    u = np.arange(n)[:, None].astype(np.float64)
    x = np.arange(n)[None, :].astype(np.float64)
    m = np.cos((2 * x + 1) * u * np.pi / (2 * n))
    scale = np.where(u == 0, np.sqrt(1.0 / n), np.sqrt(2.0 / n))
    return (m * scale).astype(np.float32)


def _build_dt1(dct: np.ndarray) -> np.ndarray:
    """DT1[8*b + x, 16*u + b] = dct[u, x], zero elsewhere."""
    dt1 = np.zeros((128, 128), dtype=np.float32)
    for b in range(16):
        dt1[8 * b : 8 * b + 8, b::16] = dct.T
    return dt1


def build_nc(
    n_img: int,
    in_engine=IN_ENGINE,
    out_engines=OUT_ENGINES,
    group=GROUP,
    udrain_engines=UDRAIN_ENGINES,
    zdrain_engines=ZDRAIN_ENGINES,
    zimg_bufs=ZIMG_BUFS,
    xs_bufs=XS_BUFS,
    ps_bufs=PS_BUFS,
):
    import concourse.bacc as bacc
    import concourse.mybir as mybir
    import concourse.tile as tile

    f16 = mybir.dt.float16
    f32 = mybir.dt.float32
    nc = bacc.Bacc("TRN2", target_bir_lowering=False, debug=False)

    x = nc.dram_tensor("x", [n_img, 128, 8 * _W], f16, kind="ExternalInput")
    dt1h = nc.dram_tensor("dt1h", [128, 128], f16, kind="ExternalInput")
    out = nc.dram_tensor("out", [n_img, 128, 8 * _W], f16, kind="ExternalOutput")

    def eng(ch):
        return {"s": nc.sync, "c": nc.scalar, "g": nc.gpsimd, "v": nc.vector}[ch]

    def copy_on(ch, dst, src):
        if ch == "v":
            nc.vector.tensor_copy(dst, src)
        elif ch == "c":
            nc.scalar.copy(dst, src)
        else:
            nc.gpsimd.copy(dst, src)

    n_out_dma = 0
    n_udrain = 0
    n_zdrain = 0

    with tile.TileContext(nc) as tc:
        with (
            tc.tile_pool(name="const", bufs=1) as constp,
            tc.tile_pool(name="xs", bufs=xs_bufs) as xsp,
            tc.tile_pool(name="zimg", bufs=zimg_bufs) as zp,
            tc.tile_pool(name="u16", bufs=3) as u16p,
            tc.tile_pool(name="psu", bufs=ps_bufs, space="PSUM") as psu,
            tc.tile_pool(name="psz", bufs=ps_bufs, space="PSUM") as psz,
        ):
            dt1h_t = constp.tile([128, 128], f16)
            nc.sync.dma_start(dt1h_t[:], dt1h[:])

            for img in range(n_img):
                # split loads/stores so compute overlaps at sub-image grain
                n_in = 4 if img == 0 else 2
                n_out = 4 if img == n_img - 1 else 2
                xs = xsp.tile([128, 8 * _W], f16)
                for k in range(n_in):
                    lo, hi = k * (8 // n_in) * _W, (k + 1) * (8 // n_in) * _W
                    eng(in_engine).dma_start(xs[:, lo:hi], x[img, :, lo:hi])

                # zimg[p=16u+bi, (s*8+tj)*128 + 16v + bj]
                zimg = zp.tile([128, 8 * _W], f16)

                for s in range(8):
                    for tj0 in range(0, 8, group):
                        gw = group * 128
                        u_ps = psu.tile([128, gw], f32)
                        for q in range(group):
                            tj = tj0 + q
                            nc.tensor.matmul(
                                u_ps[:, q * 128 : (q + 1) * 128],
                                xs[:, s * 1024 + tj * 128 : s * 1024 + (tj + 1) * 128],
                                dt1h_t[:],
                                start=True,
                                stop=True,
                            )
                        u16 = u16p.tile([128, gw], f16)
                        copy_on(
                            udrain_engines[n_udrain % len(udrain_engines)],
                            u16[:],
                            u_ps[:],
                        )
                        n_udrain += 1

                        z_ps = psz.tile([128, gw], f32)
                        for q in range(group):
                            nc.tensor.matmul(
                                z_ps[:, q * 128 : (q + 1) * 128],
                                u16[:, q * 128 : (q + 1) * 128],
                                dt1h_t[:],
                                start=True,
                                stop=True,
                            )

                        base = (s * 8 + tj0) * 128
                        copy_on(
                            zdrain_engines[n_zdrain % len(zdrain_engines)],
                            zimg[:, base : base + gw],
                            z_ps[:],
                        )
                        n_zdrain += 1

                    # ship completed zimg column ranges while later strips
                    # are still computing
                    per_out = 8 // n_out
                    if (s + 1) % per_out == 0:
                        lo = (s + 1 - per_out) * _W
                        hi = (s + 1) * _W
                        e = out_engines[n_out_dma % len(out_engines)]
                        n_out_dma += 1
                        eng(e).dma_start(out[img, :, lo:hi], zimg[:, lo:hi])

    nc.compile()
    return nc


def _get_nc(n_img: int):
    if n_img not in _NC_CACHE:
        _NC_CACHE[n_img] = build_nc(n_img)
    return _NC_CACHE[n_img]


def _prep_x(x_core: np.ndarray) -> np.ndarray:
    """[per,1,1024,1024] fp32 -> [per,128,8192] fp16 in SBUF layout."""
    per = x_core.shape[0]
    x16 = x_core.astype(np.float16)
    # rows r = 64b + 8s + x -> partition p = 8b+x, free = s*1024 + c
    xp = (
        x16.reshape(per, 16, 8, 8, _W)
        .transpose(0, 1, 3, 2, 4)
        .reshape(per, 128, 8 * _W)
    )
    return np.ascontiguousarray(xp)


def _unprep_out(od: np.ndarray) -> np.ndarray:
    """[per,128,8192] fp16 raw dump -> [per,64,128,128] fp32."""
    per = od.shape[0]
    # od[img, (u,bi), (s,t,v,bj)] -> out[img, 8u+v, 8bi+s, 16t+bj]
    o = (
        od.reshape(per, 8, 16, 8, 8, 8, 16)
        .transpose(0, 1, 5, 2, 3, 4, 6)
        .reshape(per, 64, 128, 128)
    )
    return o.astype(np.float32)


def make_inputs(x_core: np.ndarray, dct: np.ndarray) -> dict:
    dt1 = _build_dt1(dct)
    return {"x": _prep_x(x_core), "dt1h": dt1.astype(np.float16)}


def run_spmd(x: np.ndarray, dct: np.ndarray, trace: bool = False, nc=None):
    """Run the SPMD kernel on 8 cores. Returns (out, BassKernelResults)."""
    from concourse.bass_utils import run_bass_kernel_spmd

    x = np.ascontiguousarray(np.asarray(x, dtype=np.float32))
    dct = np.asarray(dct, dtype=np.float32)
    b = x.shape[0]
    per = b // _N_CORES

    if nc is None:
        nc = _get_nc(per)
    in_maps = [
        make_inputs(x[i * per : (i + 1) * per], dct) for i in range(_N_CORES)
    ]
    res = run_bass_kernel_spmd(
        nc, in_maps, core_ids=list(range(_N_CORES)), trace=trace
    )
    out = np.concatenate(
        [_unprep_out(res.results[i]["out"]) for i in range(_N_CORES)], axis=0
    )
    return out, res


def kernel(x, dct=None):
    if dct is None:
        dct = _dct_mat_np()
    out, _ = run_spmd(x, dct, trace=False)
    return out


# revision 12
# speedup vs baseline: 1.2272x; 1.0064x over previous
"""8x8 block DCT (DCT-II) on [64,1,1024,1024] fp32 -> [64,64,128,128].

Data parallel over batch: 8 images per NeuronCore on 8 cores.

fp16 end-to-end pipeline (harness gate is rel_err < 2e-2; measured ~7e-4):
  - host casts x to fp16 and pre-permutes rows/cols into the SBUF layout
    [128, 8192]: x_dram[img, p=8b+x, s*1024 + c] = x[img, 0, 64b+8s+x, c]
    so the input DMA is one fully contiguous 2 MB transfer per image
  - stage 1: U = T^T @ DT1h, one fp16 matmul per 128x128 tile
    (DT1[8*b + x, 16*u + b] = M[u, x]; contraction over partitions)
  - drain U from PSUM to fp16 SBUF (also the stage-2 operand cast)
  - stage 2: Z = U^T @ DT1h, one fp16 matmul per tile
  - drain Z from PSUM contiguously into zimg[p=16u+bi,
    (s*8+tj)*128 + 16v + bj]; one contiguous 2 MB store per image
  - host un-permutes the raw dump to [64, 128, 128] and casts to fp32

All DMAs are dense/contiguous (16 KB per-partition descriptors); the
per-core HBM traffic is 16 MB in + 16 MB out -> ~90 us roofline.
"""

import numpy as np

_N_CORES = 8
_H = 1024
_W = 1024

_NC_CACHE = {}

# tuning knobs
IN_ENGINE = "s"  # DMA descriptor-gen path: s=sync(HWDGE), c=scalar, g=gpsimd
OUT_ENGINES = "g"
GROUP = 4  # tiles per PSUM bank group (must divide 8)
UDRAIN_ENGINES = "vc"  # PSUM->SBUF fp16 drain of U (v=vector, c=scalar, g=gpsimd)
ZDRAIN_ENGINES = "cv"  # PSUM->SBUF fp16 drain of Z
ZIMG_BUFS = 3
XS_BUFS = 3
PS_BUFS = 3


def _dct_mat_np():
    n = 8
    u = np.arange(n)[:, None].astype(np.float64)
    x = np.arange(n)[None, :].astype(np.float64)
    m = np.cos((2 * x + 1) * u * np.pi / (2 * n))
    scale = np.where(u == 0, np.sqrt(1.0 / n), np.sqrt(2.0 / n))
    return (m * scale).astype(np.float32)


def _build_dt1(dct: np.ndarray) -> np.ndarray:
    """DT1[8*b + x, 16*u + b] = dct[u, x], zero elsewhere."""
    dt1 = np.zeros((128, 128), dtype=np.float32)
    for b in range(16):
        dt1[8 * b : 8 * b + 8, b::16] = dct.T
    return dt1


def build_nc(
    n_img: int,
    in_engine=IN_ENGINE,
    out_engines=OUT_ENGINES,
    group=GROUP,
    udrain_engines=UDRAIN_ENGINES,
    zdrain_engines=ZDRAIN_ENGINES,
    zimg_bufs=ZIMG_BUFS,
    xs_bufs=XS_BUFS,
    ps_bufs=PS_BUFS,
):
    import concourse.bacc as bacc
    import concourse.mybir as mybir
    import concourse.tile as tile

    f16 = mybir.dt.float16
    f32 = mybir.dt.float32
    nc = bacc.Bacc("TRN2", target_bir_lowering=False, debug=False)

    x = nc.dram_tensor("x", [n_img, 128, 8 * _W], f16, kind="ExternalInput")
    dt1h = nc.dram_tensor("dt1h", [128, 128], f16, kind="ExternalInput")
    out = nc.dram_tensor("out", [n_img, 128, 8 * _W], f16, kind="ExternalOutput")

    def eng(ch):
        return {"s": nc.sync, "c": nc.scalar, "g": nc.gpsimd, "v": nc.vector}[ch]

    def copy_on(ch, dst, src):
        if ch == "v":
            nc.vector.tensor_copy(dst, src)
        elif ch == "c":
            nc.scalar.copy(dst, src)
        else:
            nc.gpsimd.copy(dst, src)

    n_out_dma = 0
    n_udrain = 0
    n_zdrain = 0

    with tile.TileContext(nc) as tc:
        with (
            tc.tile_pool(name="const", bufs=1) as constp,
            tc.tile_pool(name="xs", bufs=xs_bufs) as xsp,
            tc.tile_pool(name="zimg", bufs=zimg_bufs) as zp,
            tc.tile_pool(name="u16", bufs=3) as u16p,
            tc.tile_pool(name="psu", bufs=ps_bufs, space="PSUM") as psu,
            tc.tile_pool(name="psz", bufs=ps_bufs, space="PSUM") as psz,
        ):
            dt1h_t = constp.tile([128, 128], f16)
            nc.sync.dma_start(dt1h_t[:], dt1h[:])

            for img in range(n_img):
                # split loads/stores so compute overlaps at sub-image grain
                n_in = 8 if img == 0 else 2
                n_out = 8 if img == n_img - 1 else 2
                xs = xsp.tile([128, 8 * _W], f16)
                for k in range(n_in):
                    lo, hi = k * (8 // n_in) * _W, (k + 1) * (8 // n_in) * _W
                    eng(in_engine).dma_start(xs[:, lo:hi], x[img, :, lo:hi])

                # zimg[p=16u+bi, (s*8+tj)*128 + 16v + bj]
                zimg = zp.tile([128, 8 * _W], f16)

                for s in range(8):
                    for tj0 in range(0, 8, group):
                        gw = group * 128
                        u_ps = psu.tile([128, gw], f32)
                        for q in range(group):
                            tj = tj0 + q
                            nc.tensor.matmul(
                                u_ps[:, q * 128 : (q + 1) * 128],
                                xs[:, s * 1024 + tj * 128 : s * 1024 + (tj + 1) * 128],
                                dt1h_t[:],
                                start=True,
                                stop=True,
                            )
                        u16 = u16p.tile([128, gw], f16)
                        copy_on(
                            udrain_engines[n_udrain % len(udrain_engines)],
                            u16[:],
                            u_ps[:],
                        )
                        n_udrain += 1

                        z_ps = psz.tile([128, gw], f32)
                        for q in range(group):
                            nc.tensor.matmul(
                                z_ps[:, q * 128 : (q + 1) * 128],
                                u16[:, q * 128 : (q + 1) * 128],
                                dt1h_t[:],
                                start=True,
                                stop=True,
                            )

                        base = (s * 8 + tj0) * 128
                        copy_on(
                            zdrain_engines[n_zdrain % len(zdrain_engines)],
                            zimg[:, base : base + gw],
                            z_ps[:],
                        )
                        n_zdrain += 1

                    # ship completed zimg column ranges while later strips
                    # are still computing
                    per_out = 8 // n_out
                    if (s + 1) % per_out == 0:
                        lo = (s + 1 - per_out) * _W
                        hi = (s + 1) * _W
                        if img == n_img - 1:
                            # tail: sync HWDGE ring (idle by now), avoids the
                            # slow SWDGE engine-15 straggler on final stores
                            e = "s"
                        else:
                            e = out_engines[n_out_dma % len(out_engines)]
                        n_out_dma += 1
                        eng(e).dma_start(out[img, :, lo:hi], zimg[:, lo:hi])

    nc.compile()
    return nc


def _get_nc(n_img: int):
    if n_img not in _NC_CACHE:
        _NC_CACHE[n_img] = build_nc(n_img)
    return _NC_CACHE[n_img]


def _prep_x(x_core: np.ndarray) -> np.ndarray:
    """[per,1,1024,1024] fp32 -> [per,128,8192] fp16 in SBUF layout."""
    per = x_core.shape[0]
    x16 = x_core.astype(np.float16)
    # rows r = 64b + 8s + x -> partition p = 8b+x, free = s*1024 + c
    xp = (
        x16.reshape(per, 16, 8, 8, _W)
        .transpose(0, 1, 3, 2, 4)
        .reshape(per, 128, 8 * _W)
    )
    return np.ascontiguousarray(xp)


def _unprep_out(od: np.ndarray) -> np.ndarray:
    """[per,128,8192] fp16 raw dump -> [per,64,128,128] fp32."""
    per = od.shape[0]
    # od[img, (u,bi), (s,t,v,bj)] -> out[img, 8u+v, 8bi+s, 16t+bj]
    o = (
        od.reshape(per, 8, 16, 8, 8, 8, 16)
        .transpose(0, 1, 5, 2, 3, 4, 6)
        .reshape(per, 64, 128, 128)
    )
    return o.astype(np.float32)


def make_inputs(x_core: np.ndarray, dct: np.ndarray) -> dict:
    dt1 = _build_dt1(dct)
    return {"x": _prep_x(x_core), "dt1h": dt1.astype(np.float16)}


def run_spmd(x: np.ndarray, dct: np.ndarray, trace: bool = False, nc=None):
    """Run the SPMD kernel on 8 cores. Returns (out, BassKernelResults)."""
    from concourse.bass_utils import run_bass_kernel_spmd

    x = np.ascontiguousarray(np.asarray(x, dtype=np.float32))
    dct = np.asarray(dct, dtype=np.float32)
    b = x.shape[0]
    per = b // _N_CORES

    if nc is None:
        nc = _get_nc(per)
    in_maps = [
        make_inputs(x[i * per : (i + 1) * per], dct) for i in range(_N_CORES)
    ]
    res = run_bass_kernel_spmd(
        nc, in_maps, core_ids=list(range(_N_CORES)), trace=trace
    )
    out = np.concatenate(
        [_unprep_out(res.results[i]["out"]) for i in range(_N_CORES)], axis=0
    )
    return out, res


def kernel(x, dct=None):
    if dct is None:
        dct = _dct_mat_np()
    out, _ = run_spmd(x, dct, trace=False)
    return out


# revision 13
# speedup vs baseline: 1.2427x; 1.0126x over previous
"""8x8 block DCT (DCT-II) on [64,1,1024,1024] fp32 -> [64,64,128,128].

Data parallel over batch: 8 images per NeuronCore on 8 cores.

fp16 end-to-end pipeline (harness gate is rel_err < 2e-2; measured ~7e-4):
  - host casts x to fp16 and pre-permutes rows/cols into the SBUF layout
    [128, 8192]: x_dram[img, p=8b+x, s*1024 + c] = x[img, 0, 64b+8s+x, c]
    so the input DMA is one fully contiguous 2 MB transfer per image
  - stage 1: U = T^T @ DT1h, one fp16 matmul per 128x128 tile
    (DT1[8*b + x, 16*u + b] = M[u, x]; contraction over partitions)
  - drain U from PSUM to fp16 SBUF (also the stage-2 operand cast)
  - stage 2: Z = U^T @ DT1h, one fp16 matmul per tile
  - drain Z from PSUM contiguously into zimg[p=16u+bi,
    (s*8+tj)*128 + 16v + bj]; one contiguous 2 MB store per image
  - host un-permutes the raw dump to [64, 128, 128] and casts to fp32

All DMAs are dense/contiguous (16 KB per-partition descriptors); the
per-core HBM traffic is 16 MB in + 16 MB out -> ~90 us roofline.
"""

import numpy as np

_N_CORES = 8
_H = 1024
_W = 1024

_NC_CACHE = {}

# tuning knobs
IN_ENGINE = "s"  # DMA descriptor-gen path: s=sync(HWDGE), c=scalar, g=gpsimd
OUT_ENGINES = "g"
GROUP = 4  # tiles per PSUM bank group (must divide 8)
UDRAIN_ENGINES = "vc"  # PSUM->SBUF fp16 drain of U (v=vector, c=scalar, g=gpsimd)
ZDRAIN_ENGINES = "cv"  # PSUM->SBUF fp16 drain of Z
ZIMG_BUFS = 3
XS_BUFS = 3
PS_BUFS = 3


def _dct_mat_np():
    n = 8
    u = np.arange(n)[:, None].astype(np.float64)
    x = np.arange(n)[None, :].astype(np.float64)
    m = np.cos((2 * x + 1) * u * np.pi / (2 * n))
    scale = np.where(u == 0, np.sqrt(1.0 / n), np.sqrt(2.0 / n))
    return (m * scale).astype(np.float32)


def _build_dt1(dct: np.ndarray) -> np.ndarray:
    """DT1[8*b + x, 16*u + b] = dct[u, x], zero elsewhere."""
    dt1 = np.zeros((128, 128), dtype=np.float32)
    for b in range(16):
        dt1[8 * b : 8 * b + 8, b::16] = dct.T
    return dt1


def build_nc(
    n_img: int,
    in_engine=IN_ENGINE,
    out_engines=OUT_ENGINES,
    group=GROUP,
    udrain_engines=UDRAIN_ENGINES,
    zdrain_engines=ZDRAIN_ENGINES,
    zimg_bufs=ZIMG_BUFS,
    xs_bufs=XS_BUFS,
    ps_bufs=PS_BUFS,
):
    import concourse.bacc as bacc
    import concourse.mybir as mybir
    import concourse.tile as tile

    f16 = mybir.dt.float16
    f32 = mybir.dt.float32
    nc = bacc.Bacc("TRN2", target_bir_lowering=False, debug=False)

    x = nc.dram_tensor("x", [n_img, 128, 8 * _W], f16, kind="ExternalInput")
    dt1h = nc.dram_tensor("dt1h", [128, 128], f16, kind="ExternalInput")
    out = nc.dram_tensor("out", [n_img, 128, 8 * _W], f16, kind="ExternalOutput")

    def eng(ch):
        return {"s": nc.sync, "c": nc.scalar, "g": nc.gpsimd, "v": nc.vector}[ch]

    def copy_on(ch, dst, src):
        if ch == "v":
            nc.vector.tensor_copy(dst, src)
        elif ch == "c":
            nc.scalar.copy(dst, src)
        else:
            nc.gpsimd.copy(dst, src)

    n_out_dma = 0
    n_udrain = 0
    n_zdrain = 0

    with tile.TileContext(nc) as tc:
        with (
            tc.tile_pool(name="const", bufs=1) as constp,
            tc.tile_pool(name="xs", bufs=xs_bufs) as xsp,
            tc.tile_pool(name="zimg", bufs=zimg_bufs) as zp,
            tc.tile_pool(name="u16", bufs=3) as u16p,
            tc.tile_pool(name="psu", bufs=ps_bufs, space="PSUM") as psu,
            tc.tile_pool(name="psz", bufs=ps_bufs, space="PSUM") as psz,
        ):
            dt1h_t = constp.tile([128, 128], f16)
            nc.sync.dma_start(dt1h_t[:], dt1h[:])

            for img in range(n_img):
                # split loads/stores so compute overlaps at sub-image grain
                n_in = 8 if img == 0 else 4
                n_out = 8 if img == n_img - 1 else 4
                xs = xsp.tile([128, 8 * _W], f16)
                for k in range(n_in):
                    lo, hi = k * (8 // n_in) * _W, (k + 1) * (8 // n_in) * _W
                    eng(in_engine).dma_start(xs[:, lo:hi], x[img, :, lo:hi])

                # zimg[p=16u+bi, (s*8+tj)*128 + 16v + bj]
                zimg = zp.tile([128, 8 * _W], f16)

                for s in range(8):
                    for tj0 in range(0, 8, group):
                        gw = group * 128
                        u_ps = psu.tile([128, gw], f32)
                        for q in range(group):
                            tj = tj0 + q
                            nc.tensor.matmul(
                                u_ps[:, q * 128 : (q + 1) * 128],
                                xs[:, s * 1024 + tj * 128 : s * 1024 + (tj + 1) * 128],
                                dt1h_t[:],
                                start=True,
                                stop=True,
                            )
                        u16 = u16p.tile([128, gw], f16)
                        copy_on(
                            udrain_engines[n_udrain % len(udrain_engines)],
                            u16[:],
                            u_ps[:],
                        )
                        n_udrain += 1

                        z_ps = psz.tile([128, gw], f32)
                        for q in range(group):
                            nc.tensor.matmul(
                                z_ps[:, q * 128 : (q + 1) * 128],
                                u16[:, q * 128 : (q + 1) * 128],
                                dt1h_t[:],
                                start=True,
                                stop=True,
                            )

                        base = (s * 8 + tj0) * 128
                        copy_on(
                            zdrain_engines[n_zdrain % len(zdrain_engines)],
                            zimg[:, base : base + gw],
                            z_ps[:],
                        )
                        n_zdrain += 1

                    # ship completed zimg column ranges while later strips
                    # are still computing
                    per_out = 8 // n_out
                    if (s + 1) % per_out == 0:
                        lo = (s + 1 - per_out) * _W
                        hi = (s + 1) * _W
                        if img == n_img - 1:
                            # tail: sync HWDGE ring (idle by now), avoids the
                            # slow SWDGE engine-15 straggler on final stores
                            e = "s"
                        else:
                            e = out_engines[n_out_dma % len(out_engines)]
                        n_out_dma += 1
                        eng(e).dma_start(out[img, :, lo:hi], zimg[:, lo:hi])

    nc.compile()
    return nc


def _get_nc(n_img: int):
    if n_img not in _NC_CACHE:
        _NC_CACHE[n_img] = build_nc(n_img)
    return _NC_CACHE[n_img]


def _prep_x(x_core: np.ndarray) -> np.ndarray:
    """[per,1,1024,1024] fp32 -> [per,128,8192] fp16 in SBUF layout."""
    per = x_core.shape[0]
    x16 = x_core.astype(np.float16)
    # rows r = 64b + 8s + x -> partition p = 8b+x, free = s*1024 + c
    xp = (
        x16.reshape(per, 16, 8, 8, _W)
        .transpose(0, 1, 3, 2, 4)
        .reshape(per, 128, 8 * _W)
    )
    return np.ascontiguousarray(xp)


def _unprep_out(od: np.ndarray) -> np.ndarray:
    """[per,128,8192] fp16 raw dump -> [per,64,128,128] fp32."""
    per = od.shape[0]
    # od[img, (u,bi), (s,t,v,bj)] -> out[img, 8u+v, 8bi+s, 16t+bj]
    o = (
        od.reshape(per, 8, 16, 8, 8, 8, 16)
        .transpose(0, 1, 5, 2, 3, 4, 6)
        .reshape(per, 64, 128, 128)
    )
    return o.astype(np.float32)


def make_inputs(x_core: np.ndarray, dct: np.ndarray) -> dict:
    dt1 = _build_dt1(dct)
    return {"x": _prep_x(x_core), "dt1h": dt1.astype(np.float16)}


def run_spmd(x: np.ndarray, dct: np.ndarray, trace: bool = False, nc=None):
    """Run the SPMD kernel on 8 cores. Returns (out, BassKernelResults)."""
    from concourse.bass_utils import run_bass_kernel_spmd

    x = np.ascontiguousarray(np.asarray(x, dtype=np.float32))
    dct = np.asarray(dct, dtype=np.float32)
    b = x.shape[0]
    per = b // _N_CORES

    if nc is None:
        nc = _get_nc(per)
    in_maps = [
        make_inputs(x[i * per : (i + 1) * per], dct) for i in range(_N_CORES)
    ]
    res = run_bass_kernel_spmd(
        nc, in_maps, core_ids=list(range(_N_CORES)), trace=trace
    )
    out = np.concatenate(
        [_unprep_out(res.results[i]["out"]) for i in range(_N_CORES)], axis=0
    )
    return out, res


def kernel(x, dct=None):
    if dct is None:
        dct = _dct_mat_np()
    out, _ = run_spmd(x, dct, trace=False)
    return out
